# revision 62
# baseline (speedup 1.0000x reference)
"""Trainium2 Bass kernel for nn_NonLocalBlock (multi-head non-local attention
block with conv/BN/SE tail).

Sharding: 8 cores = 2 batches x 4 query(o)-slices of 1024. Each core computes
full attention (all 4 heads, full key length 4096) for its o-slice, the conv
stack on its slice, and joins the SE squeeze via direct peer-SBUF remote DMA
(XOR-slot exchange within each 4-core batch group; an early overlapped
AllGather acts as the entry barrier for semaphore-preamble safety).

Math notes:
 - softmax(x) computed as exp(x/8) normalized AFTER the PV matmul: an extra
   ones-row appended to V^T gives the row sums in the same matmul (M=65).
 - exp is safe un-maxed: logits are O(5), fp32 exp handles it.
 - K bias bk is softmax-invariant ((Q+bq)@bk is constant over keys) and is
   dropped entirely; only Q keeps its bias.
 - conv bias bv folds out: message = M/s + bv (softmax weights sum to 1), so
   x = (feat - bv) - M*r with (feat - bv) precomputed on host (bf16).
 - normalization r=1/s uses reciprocal_approx_fast (~18 bits, plenty).
 - BN is inference-mode: host folds to per-channel scale/shift; conv+BN+ReLU
   epilogues run on the Scalar engine (activation Relu with AP scale/bias),
   which shares its table with Exp (no table thrash).
 - chunk-0 epilogue (normalize + conv stack) is emitted interleaved with
   chunk-1 attention so Vector/Scalar epilogue work hides under PE matmuls.
"""
import numpy as np
import ml_dtypes

import concourse.bass as bass
import concourse.tile as tile
from concourse import bacc, mybir
from concourse.bass_utils import run_bass_kernel_spmd

FP32 = mybir.dt.float32
BF16 = mybir.dt.bfloat16
ALU = mybir.AluOpType
ACTF = mybir.ActivationFunctionType

C, CH, N, BS, HEADS, DH = 256, 128, 4096, 2, 4, 64
O = 1024          # per-core o-slice
OC = 512          # o-chunk
NT = N // 128     # 32 i-tiles
NCHUNK = O // OC
EPS = 1e-5

_CACHE = {}


def _build(dbg=False):
    nc = bacc.Bacc(None, target_bir_lowering=False, debug=False)

    di = {}
    def inp(name, shape, dt):
        di[name] = nc.dram_tensor(name, list(shape), dt, kind="ExternalInput")
        return di[name]

    feat_bf = inp("feat_bf", [C, N], BF16)
    feat_bv4 = inp("feat_bv4", [64, HEADS * O], BF16)
    wq_t = inp("wq_t", [C, C], BF16)
    wk_t = inp("wk_t", [C, C], BF16)
    wv_t = inp("wv_t", [C, C], BF16)
    bq2 = inp("bq2", [128, 2], FP32)
    w1_t = inp("w1_t", [C, CH], BF16)
    w2_t = inp("w2_t", [CH, CH], BF16)
    w3_t = inp("w3_t", [CH, C], BF16)
    bn1_s = inp("bn1_s", [128, 1], FP32)
    bn1_b = inp("bn1_b", [128, 1], FP32)
    bn2_s = inp("bn2_s", [128, 1], FP32)
    bn2_b = inp("bn2_b", [128, 1], FP32)
    b3_2 = inp("b3_2", [128, 2], FP32)
    wse1_t = inp("wse1_t", [C, 16], BF16)
    wse2_t = inp("wse2_t", [16, C], BF16)
    bse1 = inp("bse1", [16, 1], FP32)
    bse2_2 = inp("bse2_2", [128, 2], FP32)

    out_d = nc.dram_tensor("out", [C, O], FP32, kind="ExternalOutput")
    dbg_d = {}
    if dbg:
        for nm, shape, dt in [
            ("dbg_q", [128, 1024], BF16),
            ("dbg_k", [128, 1024], BF16),
            ("dbg_vt", [128, 520], BF16),
            ("dbg_pv", [128, OC], FP32),
            ("dbg_rs4", [4, OC], FP32),
            ("dbg_rr", [4, OC], FP32),
            ("dbg_rb", [64, 4 * OC], FP32),
            ("dbg_x", [64, 4 * OC], BF16),
            ("dbg_h1", [128, OC], BF16),
            ("dbg_ps1", [128, OC], FP32),
            ("dbg_x2", [64, 4 * OC], BF16),
            ("dbg_x2b", [128, 2 * OC], BF16),
            ("dbg_msb", [128, 4 * OC], BF16),
            ("dbg_r0", [1, 4 * OC], FP32),
            ("dbg_msg", [128, 2 * O], FP32),
            ("dbg_sqp", [128, 4], FP32),
            ("dbg_sqg", [128, 16], FP32),
            ("dbg_gate", [128, 2], FP32),
        ]:
            dbg_d[nm] = nc.dram_tensor(nm, shape, dt, kind="ExternalOutput")

    rsem_waiters = []
    with tile.TileContext(nc) as tc:
        with (
            tc.tile_pool(name="const", bufs=1) as cpool,
            tc.tile_pool(name="work", bufs=2) as wpool,
            tc.tile_pool(name="et", bufs=3) as epool,
            tc.tile_pool(name="psA", bufs=2, space="PSUM") as psA,
            tc.tile_pool(name="psB", bufs=4, space="PSUM") as psB,
            tc.tile_pool(name="dram", bufs=1, space="DRAM") as dpool,
        ):
            # ---------------- load inputs (critical-path order) ------------
            def load(dram, shape, dt=None, name=None):
                t = cpool.tile(list(shape), dt or dram.dtype, tag=name)
                nc.sync.dma_start(t[:], dram[:])
                return t

            def load2(dram, cols, name):
                t = cpool.tile([128, 2 * cols], dram.dtype, tag=name)
                for ct in range(2):
                    nc.sync.dma_start(t[:, ct * cols:(ct + 1) * cols],
                                      dram[ct * 128:(ct + 1) * 128, :])
                return t

            sb_wq = load2(wq_t, C, "wq")      # [128, 2*256] lhsT ch-tiles
            sb_wk = load2(wk_t, C, "wk")
            sb_wv = load2(wv_t, C, "wv")
            sb_bq2 = load(bq2, [128, 2], name="bq2")
            sb_featbf = cpool.tile([128, 2 * N], BF16, tag="featbf")
            for q4 in range(4):
                for ct in range(2):
                    nc.sync.dma_start(
                        sb_featbf[:, ct * N + q4 * 1024: ct * N + (q4 + 1) * 1024],
                        feat_bf[ct * 128:(ct + 1) * 128,
                                q4 * 1024:(q4 + 1) * 1024])
            sb_featbv4 = load(feat_bv4, [64, HEADS * O], name="featbv4")
            sb_w1 = load2(w1_t, CH, "w1")
            sb_w2 = load(w2_t, [128, CH], name="w2")
            sb_w3 = load(w3_t, [128, C], name="w3")
            sb_wse1 = load2(wse1_t, 16, "wse1")
            sb_wse2 = load(wse2_t, [16, C], name="wse2")
            sb_bn1s = load(bn1_s, [128, 1], name="bn1s")
            sb_bn1b = load(bn1_b, [128, 1], name="bn1b")
            sb_bn2s = load(bn2_s, [128, 1], name="bn2s")
            sb_bn2b = load(bn2_b, [128, 1], name="bn2b")
            sb_b32 = load(b3_2, [128, 2], name="b32")
            sb_bse1 = load(bse1, [16, 1], name="bse1")
            sb_bse22 = load(bse2_2, [128, 2], name="bse22")

            # Entry barrier: a tiny AllGather early in the kernel, overlapped
            # with projections/attention. Guarantees every peer's semaphore
            # preamble has run before any remote_dma write can land.
            bar_in = dpool.tile([128, 1], FP32)
            bar_out = dpool.tile([512, 1], FP32)
            nc.sync.dma_start(bar_in[:], sb_bq2[:, 0:1])
            nc.gpsimd.collective_compute(
                "AllGather", ALU.bypass,
                replica_groups=[[0, 1, 2, 3], [4, 5, 6, 7]],
                ins=[bar_in.opt()], outs=[bar_out.opt()])
            # A GpSimd op that consumes the barrier output: since the GpSimd
            # queue is in-order, every later remote-DMA prep/trigger on it is
            # fenced behind the barrier completing.
            bar_sb = cpool.tile([128, 1], FP32, tag="barsb")
            nc.sync.dma_start(bar_sb[:], bar_out[0:128, :])
            bar_dummy = cpool.tile([4, 1], FP32, tag="bardum")
            nc.gpsimd.partition_broadcast(bar_dummy[0:4, 0:1], bar_sb[0:1, 0:1])

            # SE-squeeze exchange buffers: slot d of chunk c receives the
            # partial sums of peer (self XOR d); written by peers' remote DMA.
            rsem = nc.alloc_semaphore("sq_rsem")
            lsem = nc.alloc_semaphore("sq_lsem")
            g_sb = cpool.tile([128, 4 * 2 * NCHUNK], FP32, tag="gsb")

            O0 = 0  # o-slice offset within sb_featbf columns (host pre-slices)

            # ---------------- projections ----------------
            # Q/K psum tiles hold channels [ct*128,(ct+1)*128] = heads 2ct,2ct+1.
            # Even head's rows (0:64) / odd head's rows (64:128) go straight
            # into the dup tensors (partition-aligned); DMA mirrors the other
            # half of each.
            q_dup = cpool.tile([128, HEADS * O], BF16, tag="qdup")
            k_dup = cpool.tile([128, HEADS * N], BF16, tag="kdup")
            for ct in range(2):
                he, ho = 2 * ct, 2 * ct + 1
                ps = psA.tile([128, O], FP32, tag="s")
                for ch in range(2):
                    for half in range(2):
                        nc.tensor.matmul(
                            ps[:, half * 512:(half + 1) * 512],
                            sb_wq[:, ch * C + ct * 128: ch * C + (ct + 1) * 128],
                            sb_featbf[:, ch * N + O0 + half * 512:
                                      ch * N + O0 + half * 512 + 512],
                            start=(ch == 0), stop=(ch == 1))
                # Q bias via scalar engine (Identity: out = in + bias)
                nc.scalar.activation(q_dup[0:64, he * O:(he + 1) * O],
                                     ps[0:64, :], ACTF.Identity,
                                     bias=sb_bq2[0:64, ct:ct + 1])
                nc.scalar.activation(q_dup[64:128, ho * O:(ho + 1) * O],
                                     ps[64:128, :], ACTF.Identity,
                                     bias=sb_bq2[64:128, ct:ct + 1])
                for oc4 in range(4):
                    psk = psA.tile([128, 1024], FP32, tag="s")
                    for ch in range(2):
                        for half in range(2):
                            nc.tensor.matmul(
                                psk[:, half * 512:(half + 1) * 512],
                                sb_wk[:, ch * C + ct * 128: ch * C + (ct + 1) * 128],
                                sb_featbf[:, ch * N + oc4 * 1024 + half * 512:
                                           ch * N + oc4 * 1024 + half * 512 + 512],
                                start=(ch == 0), stop=(ch == 1))
                    # K bias dropped (softmax-invariant). Cast psum->bf16,
                    # splitting between scalar and vector engines.
                    nc.scalar.activation(
                        k_dup[0:64, he * N + oc4 * 1024: he * N + (oc4 + 1) * 1024],
                        psk[0:64, :], ACTF.Identity)
                    nc.vector.tensor_copy(
                        k_dup[64:128, ho * N + oc4 * 1024: ho * N + (oc4 + 1) * 1024],
                        psk[64:128, :])
            for h in range(4):
                if h % 2 == 0:
                    nc.sync.dma_start(q_dup[64:128, h * O:(h + 1) * O],
                                      q_dup[0:64, h * O:(h + 1) * O])
                    nc.sync.dma_start(k_dup[64:128, h * N:(h + 1) * N],
                                      k_dup[0:64, h * N:(h + 1) * N])
                else:
                    nc.sync.dma_start(q_dup[0:64, h * O:(h + 1) * O],
                                      q_dup[64:128, h * O:(h + 1) * O])
                    nc.sync.dma_start(k_dup[0:64, h * N:(h + 1) * N],
                                      k_dup[64:128, h * N:(h + 1) * N])

            # V^T with ones column: [128, NT * 260]; block (it, h) at
            # cols it*260 + h*65: cols 0-63 = V, col 64 = 1.0, so the PV
            # matmul (M=65) produces the softmax row sums in psum row 64.
            vt = cpool.tile([128, NT * 260], BF16, tag="vt")
            ones_view = vt[:].rearrange("p (i k) -> p i k", k=65)[:, :, 64:65]
            nc.vector.memset(ones_view, 1.0)
            for it in range(NT):
                ps = psB.tile([128, 256], FP32, tag="pv")
                for ch in range(2):
                    nc.tensor.matmul(
                        ps[:],
                        sb_featbf[:, ch * N + it * 128: ch * N + it * 128 + 128],
                        sb_wv[:, ch * C:(ch + 1) * C],
                        start=(ch == 0), stop=(ch == 1))
                dst = vt[:, it * 260:(it + 1) * 260] \
                    .rearrange("p (h k) -> p h k", k=65)[:, :, 0:64]
                src = ps[:].rearrange("p (h k) -> p h k", k=64)
                if it % 2 == 0:
                    nc.vector.tensor_copy(dst, src)
                else:
                    nc.scalar.activation(dst, src, ACTF.Identity)

            # ---------------- attention + conv, pipelined over chunks -------
            msg_sb = cpool.tile([128, 2 * O], FP32, tag="msg")   # conv3 out
            sq_parts = cpool.tile([128, 2 * NCHUNK], FP32, tag="sqp")

            pv_lists = [None] * NCHUNK
            epi_state = [None] * NCHUNK

            def attn_head(oc, h):
                oco = oc * OC
                pv = pv_lists[oc][h]

                def emit_pv(tp, et):
                    i0, i1 = 2 * tp, 2 * tp + 1
                    nc.tensor.matmul(
                        pv[0:65, :],
                        vt[:, i0 * 260 + h * 65: i0 * 260 + h * 65 + 65],
                        et[:, 0:OC],
                        start=(tp == 0), stop=False)
                    nc.tensor.matmul(
                        pv[0:65, :],
                        vt[:, i1 * 260 + h * 65: i1 * 260 + h * 65 + 65],
                        et[:, OC:2 * OC],
                        start=False, stop=(tp == NT // 2 - 1))

                for tp in range(NT // 2):
                    i0, i1 = 2 * tp, 2 * tp + 1
                    sps = psA.tile([128, 2 * OC], FP32, tag="s")
                    nc.tensor.matmul(
                        sps[:, 0:OC],
                        k_dup[0:64, h * N + i0 * 128: h * N + (i0 + 1) * 128],
                        q_dup[0:64, h * O + oco: h * O + oco + OC],
                        start=True, stop=True, tile_position=(0, 0))
                    nc.tensor.matmul(
                        sps[:, OC:2 * OC],
                        k_dup[64:128, h * N + i1 * 128: h * N + (i1 + 1) * 128],
                        q_dup[64:128, h * O + oco: h * O + oco + OC],
                        start=True, stop=True, tile_position=(64, 0))
                    et = epool.tile([128, 2 * OC], BF16, tag="et")
                    # exp stays on the Scalar engine: offloading half to a
                    # DVE Schraudolph bit-trick was tried and measured SLOWER
                    # (PSUM port contention with the PE).
                    nc.scalar.activation(et[:], sps[:], ACTF.Exp, scale=0.125)
                    emit_pv(tp, et)

            def epi_norm_evict(oc, heads, first=False):
                """Evict M rows (bf16) + rowsum rows (fp32) for `heads`.

                For the last (exposed) chunk the psum evictions alternate
                between Vector and Scalar so the serial chain halves; hidden
                chunks keep everything off the exp-saturated Scalar engine.
                """
                last = oc == NCHUNK - 1
                pv_list = pv_lists[oc]
                if first:
                    m_sb = wpool.tile([128, 4 * OC], BF16, tag="msb")
                    rs = wpool.tile([128, 4 * OC], FP32, tag="rs")
                    epi_state[oc] = (m_sb, rs)
                m_sb, rs = epi_state[oc]
                for h in heads:
                    dst = m_sb[0:64, h * OC:(h + 1) * OC]
                    if last and h % 2 == 1:
                        nc.scalar.activation(dst, pv_list[h][0:64, :],
                                             ACTF.Identity)
                    else:
                        nc.vector.tensor_copy(dst, pv_list[h][0:64, :])
                    dst = rs[64:65, h * OC:(h + 1) * OC]
                    if last and h % 2 == 0:
                        nc.scalar.activation(dst, pv_list[h][64:65, :],
                                             ACTF.Identity)
                    else:
                        nc.vector.tensor_copy(dst, pv_list[h][64:65, :])

            def epi_norm_r(oc):
                """r = 1/rowsum, broadcast to 64 partitions (fp32: bf16
                partition-collapse DMAs corrupt data)."""
                m_sb, rs = epi_state[oc]
                rs4 = wpool.tile([4, OC], FP32, tag="rs4")
                nc.sync.dma_start(rs4[0:4, :], rs[64:65, :])
                rr = wpool.tile([4, OC], FP32, tag="rr")
                nc.vector.reciprocal_approx_fast(rr[0:4, :], rs4[0:4, :])
                r0 = wpool.tile([1, 4 * OC], FP32, tag="r0")
                nc.sync.dma_start(r0[0:1, :], rr[0:4, :])
                rb = wpool.tile([64, 4 * OC], FP32, tag="rb")
                nc.gpsimd.partition_broadcast(rb[:], r0[0:1, :])
                if dbg and oc == 0:
                    nc.sync.dma_start(dbg_d["dbg_rs4"][:], rs4[:])
                    nc.sync.dma_start(dbg_d["dbg_rr"][:], rr[:])
                    nc.sync.dma_start(dbg_d["dbg_rb"][:], rb[:])
                    nc.sync.dma_start(dbg_d["dbg_msb"][:], m_sb[:])
                    nc.sync.dma_start(dbg_d["dbg_r0"][:], r0[:])
                epi_state[oc] = (m_sb, rb)

            def epi_norm(oc):
                epi_norm_evict(oc, range(4), first=True)
                epi_norm_r(oc)

            def epi_x(oc):
                """x_h = feat_bv - M_h * r per head, DMA-assembled to
                [128, 2*OC] channel layout."""
                oco = oc * OC
                m_sb, rb = epi_state[oc]
                x2 = wpool.tile([128, 2 * OC], BF16, tag="x2")
                for h in range(4):
                    ct, prow = h // 2, (h % 2) * 64
                    tmp = wpool.tile([64, OC], BF16, tag="tmp")
                    nc.vector.tensor_tensor(
                        tmp[:], m_sb[0:64, h * OC:(h + 1) * OC],
                        rb[0:64, h * OC:(h + 1) * OC], ALU.mult)
                    x_t = wpool.tile([64, OC], BF16, tag="xt")
                    nc.vector.tensor_tensor(
                        x_t[:],
                        sb_featbv4[:, h * O + oco: h * O + oco + OC],
                        tmp[:], ALU.subtract)
                    nc.sync.dma_start(
                        x2[prow:prow + 64, ct * OC:(ct + 1) * OC], x_t[:])
                epi_state[oc] = x2

            def epi_conv(oc):
                """conv1->bn->relu, conv2->bn->relu, conv3(+bias,+sq accum)."""
                oco = oc * OC
                x2 = epi_state[oc]
                ps12 = psA.tile([128, 2 * OC], FP32, tag="s")
                for ch in range(2):
                    nc.tensor.matmul(
                        ps12[:, 0:OC],
                        sb_w1[:, ch * CH:(ch + 1) * CH],
                        x2[:, ch * OC:(ch + 1) * OC],
                        start=(ch == 0), stop=(ch == 1))
                h1 = wpool.tile([128, OC], BF16, tag="h1")
                nc.scalar.activation(h1[:], ps12[:, 0:OC], ACTF.Relu,
                                     bias=sb_bn1b[:, 0:1], scale=sb_bn1s[:, 0:1])
                if dbg and oc == 0:
                    nc.sync.dma_start(dbg_d["dbg_h1"][:], h1[:])
                    ptmp = wpool.tile([128, OC], FP32, tag="ptmp")
                    nc.vector.tensor_copy(ptmp[:], ps12[:, 0:OC])
                    nc.sync.dma_start(dbg_d["dbg_ps1"][:], ptmp[:])
                    nc.sync.dma_start(dbg_d["dbg_x2b"][:], x2[:])
                nc.tensor.matmul(ps12[:, OC:2 * OC], sb_w2[:], h1[:],
                                 start=True, stop=True)
                h2 = wpool.tile([128, OC], BF16, tag="h2")
                nc.scalar.activation(h2[:], ps12[:, OC:2 * OC], ACTF.Relu,
                                     bias=sb_bn2b[:, 0:1], scale=sb_bn2s[:, 0:1])
                ps3 = psA.tile([128, 2 * OC], FP32, tag="s")
                for ct in range(2):
                    nc.tensor.matmul(ps3[:, ct * OC:(ct + 1) * OC],
                                     sb_w3[:, ct * 128:(ct + 1) * 128],
                                     h2[:], start=True, stop=True)
                for ct in range(2):
                    nc.scalar.activation(
                        msg_sb[:, ct * O + oco: ct * O + oco + OC],
                        ps3[:, ct * OC:(ct + 1) * OC], ACTF.Identity,
                        bias=sb_b32[:, ct:ct + 1],
                        accum_out=sq_parts[:, 2 * oc + ct: 2 * oc + ct + 1])

            def exchange_prep(oc):
                """Queue the descriptor preps for this chunk's squeeze
                exchange (XOR slots: slot d on receiver r holds the partial
                of core r^d, so the slot sum is the group total). Prepare-only
                semantics: the data read happens at trigger time, so preps
                can run hidden under attention."""
                for d in range(4):
                    # all 8 slots point at the same dest: dummy slots emit
                    # pathologically slow descriptors (+54us measured), while
                    # 8 duplicate 1KB writes are ~free. Dest rsem += 16.
                    nc.gpsimd.remote_dma_broadcast(
                        g_sb[:, oc * 8 + d * 2: oc * 8 + d * 2 + 2],
                        sq_parts[:, 2 * oc: 2 * oc + 2],
                        rsem, lsem,
                        rdests=[(0, d)] * 8)

            for oc in range(NCHUNK):
                pvl = []
                for _ in range(4):
                    pv = psB.tile([128, OC], FP32, tag="pv")
                    pvl.append(pv)
                pv_lists[oc] = pvl
                if oc == 0:
                    exchange_prep(0)
                for h in range(4):
                    attn_head(oc, h)
                    if oc > 0:
                        # interleave previous chunk's epilogue with this
                        # chunk's attention so it hides under PE matmuls
                        if h == 0:
                            epi_norm(oc - 1)
                        elif h == 1:
                            epi_x(oc - 1)
                        elif h == 2:
                            epi_conv(oc - 1)
                            if oc == NCHUNK - 1:
                                # hoist the last chunk's head-0..2 evictions
                                # under head-3's attention; head 3 evicts
                                # after its pv group stops
                                epi_norm_evict(oc, range(3), first=True)
                        elif h == 3:
                            # fire the previous chunk's preps, then queue
                            # this chunk's (a trigger fires every untriggered
                            # prep, so they must be emitted after it)
                            nc.gpsimd.trigger_dma(count=None)
                            exchange_prep(oc)
            epi_norm_evict(NCHUNK - 1, [3])
            epi_norm_r(NCHUNK - 1)
            epi_x(NCHUNK - 1)
            epi_conv(NCHUNK - 1)
            nc.gpsimd.trigger_dma(count=None)

            if dbg:
                nc.sync.dma_start(dbg_d["dbg_q"][:], q_dup[:, 0:1024])
                nc.sync.dma_start(dbg_d["dbg_k"][:], k_dup[:, 0:1024])
                nc.sync.dma_start(dbg_d["dbg_vt"][:], vt[:, 0:520])
                nc.sync.dma_start(dbg_d["dbg_msg"][:], msg_sb[:])
                nc.sync.dma_start(dbg_d["dbg_sqp"][:], sq_parts[:])
            # ---------------- SE gate (remote-DMA gathered squeeze) ---------
            # Each of the 2 chunk exchanges delivered 4 slot writes of +2
            # rsem increments each -> the slot-sum add waits for 16 (the wait
            # is patched on AFTER Tile scheduling: the single-core scheduling
            # sim cannot model remote increments and would deadlock).
            t8 = wpool.tile([128, 8], FP32, tag="t8")
            t8i = nc.vector.tensor_tensor(t8[:], g_sb[:, 0:8], g_sb[:, 8:16],
                                          ALU.add)
            rsem_waiters.append(t8i)
            t4 = wpool.tile([128, 4], FP32, tag="t4")
            nc.vector.tensor_tensor(t4[:], t8[:, 0:4], t8[:, 4:8], ALU.add)
            sq_t = wpool.tile([128, 2], FP32, tag="sqt")
            nc.vector.tensor_tensor(sq_t[:], t4[:, 0:2], t4[:, 2:4], ALU.add)
            sq_bf = wpool.tile([128, 2], BF16, tag="sqbf")
            nc.vector.tensor_scalar_mul(sq_bf[:], sq_t[:], 1.0 / N)

            fc_ps = psB.tile([128, 2], FP32, tag="pv")
            for ch in range(2):
                nc.tensor.matmul(fc_ps[0:16, 0:1],
                                 sb_wse1[:, ch * 16:(ch + 1) * 16],
                                 sq_bf[:, ch:ch + 1],
                                 start=(ch == 0), stop=(ch == 1))
            fc_sb = wpool.tile([16, 1], BF16, tag="fc")
            nc.vector.tensor_scalar(fc_sb[:], fc_ps[0:16, 0:1], sb_bse1[:, 0:1],
                                    0.0, ALU.add, ALU.max)

            g_ps = psB.tile([128, 2], FP32, tag="pv")
            for ct in range(2):
                nc.tensor.matmul(g_ps[:, ct:ct + 1],
                                 sb_wse2[:, ct * 128:(ct + 1) * 128],
                                 fc_sb[:], start=True, stop=True,
                                 skip_group_check=True)
            # sigmoid(x) = 1/(1+exp(-x)); bse2 negated on host so the Exp
            # bias (func(in*scale + bias)) lands as exp(-(x + bse2)).
            ge = wpool.tile([128, 2], FP32, tag="ge")
            nc.scalar.activation(ge[:], g_ps[:, 0:2], ACTF.Exp,
                                 bias=sb_bse22[:, 0:1], scale=-1.0)
            nc.vector.tensor_scalar_add(ge[:], ge[:], 1.0)
            gate = wpool.tile([128, 2], FP32, tag="gate")
            nc.vector.reciprocal(gate[:], ge[:])
            if dbg:
                nc.sync.dma_start(dbg_d["dbg_sqg"][:], g_sb[:])
                nc.sync.dma_start(dbg_d["dbg_gate"][:], gate[:])

            # out = feat + msg * gate  (residual from bf16 feat slice)
            for ct in range(2):
                nc.vector.scalar_tensor_tensor(
                    out=msg_sb[:, ct * O:(ct + 1) * O],
                    in0=msg_sb[:, ct * O:(ct + 1) * O],
                    scalar=gate[:, ct:ct + 1],
                    in1=sb_featbf[:, ct * N + O0: ct * N + O0 + O],
                    op0=ALU.mult, op1=ALU.add)
                nc.sync.dma_start(out_d[ct * 128:(ct + 1) * 128, :],
                                  msg_sb[:, ct * O:(ct + 1) * O])

    # Patch the receive-side waits now that Tile scheduling is done (the
    # scheduling sim can't model remote semaphore increments).
    for bi in rsem_waiters:
        # check=False: slots may already hold a Tile-assigned wait; the
        # generate_event_semaphores compile pass splits the overflow into
        # EventSemaphore instructions.
        bi.wait_op(rsem, 64 * NCHUNK, "sem-ge", check=False)
    nc.compile()
    return nc


def _prep_inputs(inputs):
    bf = ml_dtypes.bfloat16
    f = lambda x: np.ascontiguousarray(np.asarray(x, dtype=np.float32))
    feat = f(inputs["feat"])
    Wq, Wk, Wv = f(inputs["Wq"]), f(inputs["Wk"]), f(inputs["Wv"])
    bq, bv = f(inputs["bq"]), f(inputs["bv"])
    W1, W2, W3 = f(inputs["W1"]), f(inputs["W2"]), f(inputs["W3"])
    b1, b2, b3 = f(inputs["b1"]), f(inputs["b2"]), f(inputs["b3"])
    g1, be1, m1, v1 = f(inputs["g1"]), f(inputs["be1"]), f(inputs["m1"]), f(inputs["v1"])
    g2, be2, m2, v2 = f(inputs["g2"]), f(inputs["be2"]), f(inputs["m2"]), f(inputs["v2"])
    Wse1, Wse2 = f(inputs["Wse1"]), f(inputs["Wse2"])
    bse1, bse2 = f(inputs["bse1"]), f(inputs["bse2"])

    s1 = g1 / np.sqrt(v1 + EPS)
    sh1 = be1 - m1 * s1 + b1 * s1
    s2 = g2 / np.sqrt(v2 + EPS)
    sh2 = be2 - m2 * s2 + b2 * s2

    common = {
        "wq_t": np.ascontiguousarray(Wq.T).astype(bf),
        "wk_t": np.ascontiguousarray(Wk.T).astype(bf),
        "wv_t": np.ascontiguousarray(Wv.T).astype(bf),
        "bq2": np.ascontiguousarray(bq.reshape(2, 128).T),
        "w1_t": np.ascontiguousarray(W1.T).astype(bf),
        "w2_t": np.ascontiguousarray(W2.T).astype(bf),
        "w3_t": np.ascontiguousarray(W3.T).astype(bf),
        "bn1_s": s1.reshape(128, 1),
        "bn1_b": sh1.reshape(128, 1),
        "bn2_s": s2.reshape(128, 1),
        "bn2_b": sh2.reshape(128, 1),
        "b3_2": np.ascontiguousarray(b3.reshape(2, 128).T),
        "wse1_t": np.ascontiguousarray(Wse1.T).astype(bf),
        "wse2_t": np.ascontiguousarray(Wse2.T).astype(bf),
        "bse1": bse1.reshape(16, 1),
        "bse2_2": np.ascontiguousarray((-bse2).reshape(2, 128).T),
    }

    in_maps = []
    for core in range(8):
        b, osl = core // 4, core % 4
        o0 = osl * O
        fb = feat[b]
        m = dict(common)
        # roll the o-slice to the front so the kernel's slice offset is 0
        fb_roll = np.concatenate([fb[:, o0:], fb[:, :o0]], axis=1)
        m["feat_bf"] = fb_roll.astype(bf)
        fbv = fb[:, o0:o0 + O] - bv[:, None]
        m["feat_bv4"] = np.ascontiguousarray(
            np.concatenate([fbv[64 * h:64 * h + 64, :] for h in range(4)],
                           axis=1)).astype(bf)
        in_maps.append(m)
    return in_maps


def kernel(**inputs) -> np.ndarray:
    if "nc" not in _CACHE:
        _CACHE["nc"] = _build()
    nc = _CACHE["nc"]
    in_maps = _prep_inputs(inputs)
    res = run_bass_kernel_spmd(nc, in_maps, core_ids=list(range(8)))
    out = np.zeros((BS, C, N), dtype=np.float32)
    for core in range(8):
        b, osl = core // 4, core % 4
        out[b, :, osl * O:(osl + 1) * O] = res.results[core]["out"]
    return out


if __name__ == "__main__":
    import sys
    sys.path.insert(0, "/root/problem")
    from reference import setup_inputs, reference
    inp = {k: np.asarray(v) for k, v in setup_inputs().items()}
    ref = np.asarray(reference(**inp))
    got = kernel(**inp)
    err = np.abs(got - ref)
    print("absmax err:", err.max(), "ref absmax:", np.abs(ref).max())
    print("Relative error:", err.max() / np.abs(ref).max())


# revision 64
# speedup vs baseline: 2.2150x; 2.2150x over previous
"""Trainium2 Bass kernel for nn_NonLocalBlock (multi-head non-local attention
block with conv/BN/SE tail).

Sharding: 8 cores = 2 batches x 4 query(o)-slices of 1024. Each core computes
full attention (all 4 heads, full key length 4096) for its o-slice, the conv
stack on its slice, and joins the SE squeeze via direct peer-SBUF remote DMA
(XOR-slot exchange within each 4-core batch group; an early overlapped
AllGather acts as the entry barrier for semaphore-preamble safety).

Math notes:
 - softmax(x) computed as exp(x/8) normalized AFTER the PV matmul: an extra
   ones-row appended to V^T gives the row sums in the same matmul (M=65).
 - exp is safe un-maxed: logits are O(5), fp32 exp handles it.
 - K bias bk is softmax-invariant ((Q+bq)@bk is constant over keys) and is
   dropped entirely; only Q keeps its bias.
 - conv bias bv folds out: message = M/s + bv (softmax weights sum to 1), so
   x = (feat - bv) - M*r with (feat - bv) precomputed on host (bf16).
 - normalization r=1/s uses reciprocal_approx_fast (~18 bits, plenty).
 - BN is inference-mode: host folds to per-channel scale/shift; conv+BN+ReLU
   epilogues run on the Scalar engine (activation Relu with AP scale/bias),
   which shares its table with Exp (no table thrash).
 - chunk-0 epilogue (normalize + conv stack) is emitted interleaved with
   chunk-1 attention so Vector/Scalar epilogue work hides under PE matmuls.
"""
import numpy as np
import ml_dtypes

import concourse.bass as bass
import concourse.tile as tile
from concourse import bacc, mybir
from concourse.bass_utils import run_bass_kernel_spmd

FP32 = mybir.dt.float32
BF16 = mybir.dt.bfloat16
ALU = mybir.AluOpType
ACTF = mybir.ActivationFunctionType

C, CH, N, BS, HEADS, DH = 256, 128, 4096, 2, 4, 64
O = 1024          # per-core o-slice
OC = 512          # o-chunk
NT = N // 128     # 32 i-tiles
NCHUNK = O // OC
EPS = 1e-5

_CACHE = {}


def _build(dbg=False):
    nc = bacc.Bacc(None, target_bir_lowering=False, debug=False)

    di = {}
    def inp(name, shape, dt):
        di[name] = nc.dram_tensor(name, list(shape), dt, kind="ExternalInput")
        return di[name]

    feat_bf = inp("feat_bf", [C, N], BF16)
    feat_bv4 = inp("feat_bv4", [64, HEADS * O], BF16)
    wq_t = inp("wq_t", [C, C], BF16)
    wk_t = inp("wk_t", [C, C], BF16)
    wv_t = inp("wv_t", [C, C], BF16)
    bq2 = inp("bq2", [128, 2], FP32)
    w1_t = inp("w1_t", [C, CH], BF16)
    w2_t = inp("w2_t", [CH, CH], BF16)
    w3_t = inp("w3_t", [CH, C], BF16)
    bn1_s = inp("bn1_s", [128, 1], FP32)
    bn1_b = inp("bn1_b", [128, 1], FP32)
    bn2_s = inp("bn2_s", [128, 1], FP32)
    bn2_b = inp("bn2_b", [128, 1], FP32)
    b3_2 = inp("b3_2", [128, 2], FP32)
    wse1_t = inp("wse1_t", [C, 16], BF16)
    wse2_t = inp("wse2_t", [16, C], BF16)
    bse1 = inp("bse1", [16, 1], FP32)
    bse2_2 = inp("bse2_2", [128, 2], FP32)

    out_d = nc.dram_tensor("out", [C, O], FP32, kind="ExternalOutput")
    dbg_d = {}
    if dbg:
        for nm, shape, dt in [
            ("dbg_q", [128, 1024], BF16),
            ("dbg_k", [128, 1024], BF16),
            ("dbg_vt", [128, 520], BF16),
            ("dbg_pv", [128, OC], FP32),
            ("dbg_rs4", [4, OC], FP32),
            ("dbg_rr", [4, OC], FP32),
            ("dbg_rb", [64, 4 * OC], FP32),
            ("dbg_x", [64, 4 * OC], BF16),
            ("dbg_h1", [128, OC], BF16),
            ("dbg_ps1", [128, OC], FP32),
            ("dbg_x2", [64, 4 * OC], BF16),
            ("dbg_x2b", [128, 2 * OC], BF16),
            ("dbg_msb", [128, 4 * OC], BF16),
            ("dbg_r0", [1, 4 * OC], FP32),
            ("dbg_msg", [128, 2 * O], FP32),
            ("dbg_sqp", [128, 4], FP32),
            ("dbg_sqg", [128, 16], FP32),
            ("dbg_gate", [128, 2], FP32),
        ]:
            dbg_d[nm] = nc.dram_tensor(nm, shape, dt, kind="ExternalOutput")

    rsem_waiters = []
    with tile.TileContext(nc) as tc:
        with (
            tc.tile_pool(name="const", bufs=1) as cpool,
            tc.tile_pool(name="work", bufs=2) as wpool,
            tc.tile_pool(name="et", bufs=3) as epool,
            tc.tile_pool(name="psA", bufs=2, space="PSUM") as psA,
            tc.tile_pool(name="psB", bufs=4, space="PSUM") as psB,
            tc.tile_pool(name="dram", bufs=1, space="DRAM") as dpool,
        ):
            # ---------------- load inputs (critical-path order) ------------
            def load(dram, shape, dt=None, name=None):
                t = cpool.tile(list(shape), dt or dram.dtype, tag=name)
                nc.sync.dma_start(t[:], dram[:])
                return t

            def load2(dram, cols, name):
                t = cpool.tile([128, 2 * cols], dram.dtype, tag=name)
                for ct in range(2):
                    nc.sync.dma_start(t[:, ct * cols:(ct + 1) * cols],
                                      dram[ct * 128:(ct + 1) * 128, :])
                return t

            sb_wq = load2(wq_t, C, "wq")      # [128, 2*256] lhsT ch-tiles
            sb_wk = load2(wk_t, C, "wk")
            sb_wv = load2(wv_t, C, "wv")
            sb_bq2 = load(bq2, [128, 2], name="bq2")
            sb_featbf = cpool.tile([128, 2 * N], BF16, tag="featbf")
            for q4 in range(4):
                for ct in range(2):
                    nc.sync.dma_start(
                        sb_featbf[:, ct * N + q4 * 1024: ct * N + (q4 + 1) * 1024],
                        feat_bf[ct * 128:(ct + 1) * 128,
                                q4 * 1024:(q4 + 1) * 1024])
            sb_featbv4 = load(feat_bv4, [64, HEADS * O], name="featbv4")
            sb_w1 = load2(w1_t, CH, "w1")
            sb_w2 = load(w2_t, [128, CH], name="w2")
            sb_w3 = load(w3_t, [128, C], name="w3")
            sb_wse1 = load2(wse1_t, 16, "wse1")
            sb_wse2 = load(wse2_t, [16, C], name="wse2")
            sb_bn1s = load(bn1_s, [128, 1], name="bn1s")
            sb_bn1b = load(bn1_b, [128, 1], name="bn1b")
            sb_bn2s = load(bn2_s, [128, 1], name="bn2s")
            sb_bn2b = load(bn2_b, [128, 1], name="bn2b")
            sb_b32 = load(b3_2, [128, 2], name="b32")
            sb_bse1 = load(bse1, [16, 1], name="bse1")
            sb_bse22 = load(bse2_2, [128, 2], name="bse22")

            # Entry barrier: a tiny AllGather early in the kernel, overlapped
            # with projections/attention. Guarantees every peer's semaphore
            # preamble has run before any remote_dma write can land.
            bar_in = dpool.tile([128, 1], FP32)
            bar_out = dpool.tile([512, 1], FP32)
            nc.sync.dma_start(bar_in[:], sb_bq2[:, 0:1])
            nc.gpsimd.collective_compute(
                "AllGather", ALU.bypass,
                replica_groups=[[0, 1, 2, 3], [4, 5, 6, 7]],
                ins=[bar_in.opt()], outs=[bar_out.opt()])
            # A GpSimd op that consumes the barrier output: since the GpSimd
            # queue is in-order, every later remote-DMA prep/trigger on it is
            # fenced behind the barrier completing.
            bar_sb = cpool.tile([128, 1], FP32, tag="barsb")
            nc.sync.dma_start(bar_sb[:], bar_out[0:128, :])
            bar_dummy = cpool.tile([4, 1], FP32, tag="bardum")
            nc.gpsimd.partition_broadcast(bar_dummy[0:4, 0:1], bar_sb[0:1, 0:1])

            # SE-squeeze exchange buffers: slot d of chunk c receives the
            # partial sums of peer (self XOR d); written by peers' remote DMA.
            rsem = nc.alloc_semaphore("sq_rsem")
            lsem = nc.alloc_semaphore("sq_lsem")
            g_sb = cpool.tile([128, 4 * 2 * NCHUNK], FP32, tag="gsb")

            O0 = 0  # o-slice offset within sb_featbf columns (host pre-slices)

            # ---------------- projections ----------------
            # Q/K psum tiles hold channels [ct*128,(ct+1)*128] = heads 2ct,2ct+1.
            # Even head's rows (0:64) / odd head's rows (64:128) go straight
            # into the dup tensors (partition-aligned); DMA mirrors the other
            # half of each.
            q_dup = cpool.tile([128, HEADS * O], BF16, tag="qdup")
            k_dup = cpool.tile([128, HEADS * N], BF16, tag="kdup")
            for ct in range(2):
                he, ho = 2 * ct, 2 * ct + 1
                ps = psA.tile([128, O], FP32, tag="s")
                for ch in range(2):
                    for half in range(2):
                        nc.tensor.matmul(
                            ps[:, half * 512:(half + 1) * 512],
                            sb_wq[:, ch * C + ct * 128: ch * C + (ct + 1) * 128],
                            sb_featbf[:, ch * N + O0 + half * 512:
                                      ch * N + O0 + half * 512 + 512],
                            start=(ch == 0), stop=(ch == 1))
                # Q bias via scalar engine (Identity: out = in + bias)
                nc.scalar.activation(q_dup[0:64, he * O:(he + 1) * O],
                                     ps[0:64, :], ACTF.Identity,
                                     bias=sb_bq2[0:64, ct:ct + 1])
                nc.scalar.activation(q_dup[64:128, ho * O:(ho + 1) * O],
                                     ps[64:128, :], ACTF.Identity,
                                     bias=sb_bq2[64:128, ct:ct + 1])
                for oc4 in range(4):
                    psk = psA.tile([128, 1024], FP32, tag="s")
                    for ch in range(2):
                        for half in range(2):
                            nc.tensor.matmul(
                                psk[:, half * 512:(half + 1) * 512],
                                sb_wk[:, ch * C + ct * 128: ch * C + (ct + 1) * 128],
                                sb_featbf[:, ch * N + oc4 * 1024 + half * 512:
                                           ch * N + oc4 * 1024 + half * 512 + 512],
                                start=(ch == 0), stop=(ch == 1))
                    # K bias dropped (softmax-invariant). Cast psum->bf16,
                    # splitting between scalar and vector engines.
                    nc.scalar.activation(
                        k_dup[0:64, he * N + oc4 * 1024: he * N + (oc4 + 1) * 1024],
                        psk[0:64, :], ACTF.Identity)
                    nc.vector.tensor_copy(
                        k_dup[64:128, ho * N + oc4 * 1024: ho * N + (oc4 + 1) * 1024],
                        psk[64:128, :])
            for h in range(4):
                if h % 2 == 0:
                    nc.sync.dma_start(q_dup[64:128, h * O:(h + 1) * O],
                                      q_dup[0:64, h * O:(h + 1) * O])
                    nc.sync.dma_start(k_dup[64:128, h * N:(h + 1) * N],
                                      k_dup[0:64, h * N:(h + 1) * N])
                else:
                    nc.sync.dma_start(q_dup[0:64, h * O:(h + 1) * O],
                                      q_dup[64:128, h * O:(h + 1) * O])
                    nc.sync.dma_start(k_dup[0:64, h * N:(h + 1) * N],
                                      k_dup[64:128, h * N:(h + 1) * N])

            # V^T with ones column: [128, NT * 260]; block (it, h) at
            # cols it*260 + h*65: cols 0-63 = V, col 64 = 1.0, so the PV
            # matmul (M=65) produces the softmax row sums in psum row 64.
            vt = cpool.tile([128, NT * 260], BF16, tag="vt")
            ones_view = vt[:].rearrange("p (i k) -> p i k", k=65)[:, :, 64:65]
            nc.vector.memset(ones_view, 1.0)
            for it in range(NT):
                ps = psB.tile([128, 256], FP32, tag="pv")
                for ch in range(2):
                    nc.tensor.matmul(
                        ps[:],
                        sb_featbf[:, ch * N + it * 128: ch * N + it * 128 + 128],
                        sb_wv[:, ch * C:(ch + 1) * C],
                        start=(ch == 0), stop=(ch == 1))
                dst = vt[:, it * 260:(it + 1) * 260] \
                    .rearrange("p (h k) -> p h k", k=65)[:, :, 0:64]
                src = ps[:].rearrange("p (h k) -> p h k", k=64)
                if it % 2 == 0:
                    nc.vector.tensor_copy(dst, src)
                else:
                    nc.scalar.activation(dst, src, ACTF.Identity)

            # ---------------- attention + conv, pipelined over chunks -------
            msg_sb = cpool.tile([128, 2 * O], FP32, tag="msg")   # conv3 out
            sq_parts = cpool.tile([128, 2 * NCHUNK], FP32, tag="sqp")

            pv_lists = [None] * NCHUNK
            epi_state = [None] * NCHUNK

            def attn_head(oc, h):
                oco = oc * OC
                pv = pv_lists[oc][h]

                def emit_pv(tp, et):
                    i0, i1 = 2 * tp, 2 * tp + 1
                    nc.tensor.matmul(
                        pv[0:65, :],
                        vt[:, i0 * 260 + h * 65: i0 * 260 + h * 65 + 65],
                        et[:, 0:OC],
                        start=(tp == 0), stop=False)
                    nc.tensor.matmul(
                        pv[0:65, :],
                        vt[:, i1 * 260 + h * 65: i1 * 260 + h * 65 + 65],
                        et[:, OC:2 * OC],
                        start=False, stop=(tp == NT // 2 - 1))

                for tp in range(NT // 2):
                    i0, i1 = 2 * tp, 2 * tp + 1
                    sps = psA.tile([128, 2 * OC], FP32, tag="s")
                    nc.tensor.matmul(
                        sps[:, 0:OC],
                        k_dup[0:64, h * N + i0 * 128: h * N + (i0 + 1) * 128],
                        q_dup[0:64, h * O + oco: h * O + oco + OC],
                        start=True, stop=True, tile_position=(0, 0))
                    nc.tensor.matmul(
                        sps[:, OC:2 * OC],
                        k_dup[64:128, h * N + i1 * 128: h * N + (i1 + 1) * 128],
                        q_dup[64:128, h * O + oco: h * O + oco + OC],
                        start=True, stop=True, tile_position=(64, 0))
                    et = epool.tile([128, 2 * OC], BF16, tag="et")
                    # exp stays on the Scalar engine: offloading half to a
                    # DVE Schraudolph bit-trick was tried and measured SLOWER
                    # (PSUM port contention with the PE).
                    nc.scalar.activation(et[:], sps[:], ACTF.Exp, scale=0.125)
                    emit_pv(tp, et)

            def epi_norm_evict(oc, heads, first=False):
                """Evict M rows (bf16) + rowsum rows (fp32) for `heads`.

                For the last (exposed) chunk the psum evictions alternate
                between Vector and Scalar so the serial chain halves; hidden
                chunks keep everything off the exp-saturated Scalar engine.
                """
                last = oc == NCHUNK - 1
                pv_list = pv_lists[oc]
                if first:
                    m_sb = wpool.tile([128, 4 * OC], BF16, tag="msb")
                    rs = wpool.tile([128, 4 * OC], FP32, tag="rs")
                    epi_state[oc] = (m_sb, rs)
                m_sb, rs = epi_state[oc]
                for h in heads:
                    dst = m_sb[0:64, h * OC:(h + 1) * OC]
                    if last and h % 2 == 1:
                        nc.scalar.activation(dst, pv_list[h][0:64, :],
                                             ACTF.Identity)
                    else:
                        nc.vector.tensor_copy(dst, pv_list[h][0:64, :])
                    dst = rs[64:65, h * OC:(h + 1) * OC]
                    if last and h % 2 == 0:
                        nc.scalar.activation(dst, pv_list[h][64:65, :],
                                             ACTF.Identity)
                    else:
                        nc.vector.tensor_copy(dst, pv_list[h][64:65, :])

            def epi_norm_r(oc):
                """r = 1/rowsum, broadcast to 64 partitions (fp32: bf16
                partition-collapse DMAs corrupt data)."""
                m_sb, rs = epi_state[oc]
                rs4 = wpool.tile([4, OC], FP32, tag="rs4")
                nc.sync.dma_start(rs4[0:4, :], rs[64:65, :])
                rr = wpool.tile([4, OC], FP32, tag="rr")
                nc.vector.reciprocal_approx_fast(rr[0:4, :], rs4[0:4, :])
                r0 = wpool.tile([1, 4 * OC], FP32, tag="r0")
                nc.sync.dma_start(r0[0:1, :], rr[0:4, :])
                rb = wpool.tile([64, 4 * OC], FP32, tag="rb")
                nc.gpsimd.partition_broadcast(rb[:], r0[0:1, :])
                if dbg and oc == 0:
                    nc.sync.dma_start(dbg_d["dbg_rs4"][:], rs4[:])
                    nc.sync.dma_start(dbg_d["dbg_rr"][:], rr[:])
                    nc.sync.dma_start(dbg_d["dbg_rb"][:], rb[:])
                    nc.sync.dma_start(dbg_d["dbg_msb"][:], m_sb[:])
                    nc.sync.dma_start(dbg_d["dbg_r0"][:], r0[:])
                epi_state[oc] = (m_sb, rb)

            def epi_norm(oc):
                epi_norm_evict(oc, range(4), first=True)
                epi_norm_r(oc)

            def epi_x(oc):
                """x_h = feat_bv - M_h * r per head, DMA-assembled to
                [128, 2*OC] channel layout."""
                oco = oc * OC
                m_sb, rb = epi_state[oc]
                x2 = wpool.tile([128, 2 * OC], BF16, tag="x2")
                for h in range(4):
                    ct, prow = h // 2, (h % 2) * 64
                    tmp = wpool.tile([64, OC], BF16, tag="tmp")
                    nc.vector.tensor_tensor(
                        tmp[:], m_sb[0:64, h * OC:(h + 1) * OC],
                        rb[0:64, h * OC:(h + 1) * OC], ALU.mult)
                    x_t = wpool.tile([64, OC], BF16, tag="xt")
                    nc.vector.tensor_tensor(
                        x_t[:],
                        sb_featbv4[:, h * O + oco: h * O + oco + OC],
                        tmp[:], ALU.subtract)
                    nc.sync.dma_start(
                        x2[prow:prow + 64, ct * OC:(ct + 1) * OC], x_t[:])
                epi_state[oc] = x2

            def epi_conv(oc):
                """conv1->bn->relu, conv2->bn->relu, conv3(+bias,+sq accum)."""
                oco = oc * OC
                x2 = epi_state[oc]
                ps12 = psA.tile([128, 2 * OC], FP32, tag="s")
                for ch in range(2):
                    nc.tensor.matmul(
                        ps12[:, 0:OC],
                        sb_w1[:, ch * CH:(ch + 1) * CH],
                        x2[:, ch * OC:(ch + 1) * OC],
                        start=(ch == 0), stop=(ch == 1))
                h1 = wpool.tile([128, OC], BF16, tag="h1")
                nc.scalar.activation(h1[:], ps12[:, 0:OC], ACTF.Relu,
                                     bias=sb_bn1b[:, 0:1], scale=sb_bn1s[:, 0:1])
                if dbg and oc == 0:
                    nc.sync.dma_start(dbg_d["dbg_h1"][:], h1[:])
                    ptmp = wpool.tile([128, OC], FP32, tag="ptmp")
                    nc.vector.tensor_copy(ptmp[:], ps12[:, 0:OC])
                    nc.sync.dma_start(dbg_d["dbg_ps1"][:], ptmp[:])
                    nc.sync.dma_start(dbg_d["dbg_x2b"][:], x2[:])
                nc.tensor.matmul(ps12[:, OC:2 * OC], sb_w2[:], h1[:],
                                 start=True, stop=True)
                h2 = wpool.tile([128, OC], BF16, tag="h2")
                nc.scalar.activation(h2[:], ps12[:, OC:2 * OC], ACTF.Relu,
                                     bias=sb_bn2b[:, 0:1], scale=sb_bn2s[:, 0:1])
                ps3 = psA.tile([128, 2 * OC], FP32, tag="s")
                for ct in range(2):
                    nc.tensor.matmul(ps3[:, ct * OC:(ct + 1) * OC],
                                     sb_w3[:, ct * 128:(ct + 1) * 128],
                                     h2[:], start=True, stop=True)
                for ct in range(2):
                    nc.scalar.activation(
                        msg_sb[:, ct * O + oco: ct * O + oco + OC],
                        ps3[:, ct * OC:(ct + 1) * OC], ACTF.Identity,
                        bias=sb_b32[:, ct:ct + 1],
                        accum_out=sq_parts[:, 2 * oc + ct: 2 * oc + ct + 1])

            def exchange_prep(oc):
                """Queue the descriptor preps for this chunk's squeeze
                exchange (XOR slots: slot d on receiver r holds the partial
                of core r^d, so the slot sum is the group total). Prepare-only
                semantics: the data read happens at trigger time, so preps
                can run hidden under attention."""
                for d in range(4):
                    # all 8 slots point at the same dest: dummy slots emit
                    # pathologically slow descriptors (+54us measured), while
                    # 8 duplicate 1KB writes are ~free. Dest rsem += 16.
                    nc.gpsimd.remote_dma_broadcast(
                        g_sb[:, oc * 8 + d * 2: oc * 8 + d * 2 + 2],
                        sq_parts[:, 2 * oc: 2 * oc + 2],
                        rsem, lsem,
                        rdests=[(0, d)] * 8)

            for oc in range(NCHUNK):
                pvl = []
                for _ in range(4):
                    pv = psB.tile([128, OC], FP32, tag="pv")
                    pvl.append(pv)
                pv_lists[oc] = pvl
                for h in range(4):
                    attn_head(oc, h)
                    if oc > 0:
                        # interleave previous chunk's epilogue with this
                        # chunk's attention so it hides under PE matmuls
                        if h == 0:
                            epi_norm(oc - 1)
                        elif h == 1:
                            epi_x(oc - 1)
                        elif h == 2:
                            epi_conv(oc - 1)
                        elif h == 3:
                            # preps must be emitted AFTER the sq_parts writes:
                            # remote-DMA src deps are user-managed, so a prep
                            # emitted before the producer races (stale sends)
                            exchange_prep(oc - 1)
                            nc.gpsimd.trigger_dma(count=None)
            epi_norm(NCHUNK - 1)
            epi_x(NCHUNK - 1)
            epi_conv(NCHUNK - 1)
            exchange_prep(NCHUNK - 1)
            nc.gpsimd.trigger_dma(count=None)

            if dbg:
                nc.sync.dma_start(dbg_d["dbg_q"][:], q_dup[:, 0:1024])
                nc.sync.dma_start(dbg_d["dbg_k"][:], k_dup[:, 0:1024])
                nc.sync.dma_start(dbg_d["dbg_vt"][:], vt[:, 0:520])
                nc.sync.dma_start(dbg_d["dbg_msg"][:], msg_sb[:])
                nc.sync.dma_start(dbg_d["dbg_sqp"][:], sq_parts[:])
            # ---------------- SE gate (remote-DMA gathered squeeze) ---------
            # Each of the 2 chunk exchanges delivered 4 slot writes of +2
            # rsem increments each -> the slot-sum add waits for 16 (the wait
            # is patched on AFTER Tile scheduling: the single-core scheduling
            # sim cannot model remote increments and would deadlock).
            t8 = wpool.tile([128, 8], FP32, tag="t8")
            t8i = nc.vector.tensor_tensor(t8[:], g_sb[:, 0:8], g_sb[:, 8:16],
                                          ALU.add)
            rsem_waiters.append(t8i)
            t4 = wpool.tile([128, 4], FP32, tag="t4")
            nc.vector.tensor_tensor(t4[:], t8[:, 0:4], t8[:, 4:8], ALU.add)
            sq_t = wpool.tile([128, 2], FP32, tag="sqt")
            nc.vector.tensor_tensor(sq_t[:], t4[:, 0:2], t4[:, 2:4], ALU.add)
            sq_bf = wpool.tile([128, 2], BF16, tag="sqbf")
            nc.vector.tensor_scalar_mul(sq_bf[:], sq_t[:], 1.0 / N)

            fc_ps = psB.tile([128, 2], FP32, tag="pv")
            for ch in range(2):
                nc.tensor.matmul(fc_ps[0:16, 0:1],
                                 sb_wse1[:, ch * 16:(ch + 1) * 16],
                                 sq_bf[:, ch:ch + 1],
                                 start=(ch == 0), stop=(ch == 1))
            fc_sb = wpool.tile([16, 1], BF16, tag="fc")
            nc.vector.tensor_scalar(fc_sb[:], fc_ps[0:16, 0:1], sb_bse1[:, 0:1],
                                    0.0, ALU.add, ALU.max)

            g_ps = psB.tile([128, 2], FP32, tag="pv")
            for ct in range(2):
                nc.tensor.matmul(g_ps[:, ct:ct + 1],
                                 sb_wse2[:, ct * 128:(ct + 1) * 128],
                                 fc_sb[:], start=True, stop=True,
                                 skip_group_check=True)
            # sigmoid(x) = 1/(1+exp(-x)); bse2 negated on host so the Exp
            # bias (func(in*scale + bias)) lands as exp(-(x + bse2)).
            ge = wpool.tile([128, 2], FP32, tag="ge")
            nc.scalar.activation(ge[:], g_ps[:, 0:2], ACTF.Exp,
                                 bias=sb_bse22[:, 0:1], scale=-1.0)
            nc.vector.tensor_scalar_add(ge[:], ge[:], 1.0)
            gate = wpool.tile([128, 2], FP32, tag="gate")
            nc.vector.reciprocal(gate[:], ge[:])
            if dbg:
                nc.sync.dma_start(dbg_d["dbg_sqg"][:], g_sb[:])
                nc.sync.dma_start(dbg_d["dbg_gate"][:], gate[:])

            # out = feat + msg * gate  (residual from bf16 feat slice)
            for ct in range(2):
                nc.vector.scalar_tensor_tensor(
                    out=msg_sb[:, ct * O:(ct + 1) * O],
                    in0=msg_sb[:, ct * O:(ct + 1) * O],
                    scalar=gate[:, ct:ct + 1],
                    in1=sb_featbf[:, ct * N + O0: ct * N + O0 + O],
                    op0=ALU.mult, op1=ALU.add)
                nc.sync.dma_start(out_d[ct * 128:(ct + 1) * 128, :],
                                  msg_sb[:, ct * O:(ct + 1) * O])

    # Patch the receive-side waits now that Tile scheduling is done (the
    # scheduling sim can't model remote semaphore increments).
    for bi in rsem_waiters:
        # check=False: slots may already hold a Tile-assigned wait; the
        # generate_event_semaphores compile pass splits the overflow into
        # EventSemaphore instructions.
        bi.wait_op(rsem, 64 * NCHUNK, "sem-ge", check=False)
    nc.compile()
    return nc


def _prep_inputs(inputs):
    bf = ml_dtypes.bfloat16
    f = lambda x: np.ascontiguousarray(np.asarray(x, dtype=np.float32))
    feat = f(inputs["feat"])
    Wq, Wk, Wv = f(inputs["Wq"]), f(inputs["Wk"]), f(inputs["Wv"])
    bq, bv = f(inputs["bq"]), f(inputs["bv"])
    W1, W2, W3 = f(inputs["W1"]), f(inputs["W2"]), f(inputs["W3"])
    b1, b2, b3 = f(inputs["b1"]), f(inputs["b2"]), f(inputs["b3"])
    g1, be1, m1, v1 = f(inputs["g1"]), f(inputs["be1"]), f(inputs["m1"]), f(inputs["v1"])
    g2, be2, m2, v2 = f(inputs["g2"]), f(inputs["be2"]), f(inputs["m2"]), f(inputs["v2"])
    Wse1, Wse2 = f(inputs["Wse1"]), f(inputs["Wse2"])
    bse1, bse2 = f(inputs["bse1"]), f(inputs["bse2"])

    s1 = g1 / np.sqrt(v1 + EPS)
    sh1 = be1 - m1 * s1 + b1 * s1
    s2 = g2 / np.sqrt(v2 + EPS)
    sh2 = be2 - m2 * s2 + b2 * s2

    common = {
        "wq_t": np.ascontiguousarray(Wq.T).astype(bf),
        "wk_t": np.ascontiguousarray(Wk.T).astype(bf),
        "wv_t": np.ascontiguousarray(Wv.T).astype(bf),
        "bq2": np.ascontiguousarray(bq.reshape(2, 128).T),
        "w1_t": np.ascontiguousarray(W1.T).astype(bf),
        "w2_t": np.ascontiguousarray(W2.T).astype(bf),
        "w3_t": np.ascontiguousarray(W3.T).astype(bf),
        "bn1_s": s1.reshape(128, 1),
        "bn1_b": sh1.reshape(128, 1),
        "bn2_s": s2.reshape(128, 1),
        "bn2_b": sh2.reshape(128, 1),
        "b3_2": np.ascontiguousarray(b3.reshape(2, 128).T),
        "wse1_t": np.ascontiguousarray(Wse1.T).astype(bf),
        "wse2_t": np.ascontiguousarray(Wse2.T).astype(bf),
        "bse1": bse1.reshape(16, 1),
        "bse2_2": np.ascontiguousarray((-bse2).reshape(2, 128).T),
    }

    in_maps = []
    for core in range(8):
        b, osl = core // 4, core % 4
        o0 = osl * O
        fb = feat[b]
        m = dict(common)
        # roll the o-slice to the front so the kernel's slice offset is 0
        fb_roll = np.concatenate([fb[:, o0:], fb[:, :o0]], axis=1)
        m["feat_bf"] = fb_roll.astype(bf)
        fbv = fb[:, o0:o0 + O] - bv[:, None]
        m["feat_bv4"] = np.ascontiguousarray(
            np.concatenate([fbv[64 * h:64 * h + 64, :] for h in range(4)],
                           axis=1)).astype(bf)
        in_maps.append(m)
    return in_maps


def kernel(**inputs) -> np.ndarray:
    if "nc" not in _CACHE:
        _CACHE["nc"] = _build()
    nc = _CACHE["nc"]
    in_maps = _prep_inputs(inputs)
    res = run_bass_kernel_spmd(nc, in_maps, core_ids=list(range(8)))
    out = np.zeros((BS, C, N), dtype=np.float32)
    for core in range(8):
        b, osl = core // 4, core % 4
        out[b, :, osl * O:(osl + 1) * O] = res.results[core]["out"]
    return out


if __name__ == "__main__":
    import sys
    sys.path.insert(0, "/root/problem")
    from reference import setup_inputs, reference
    inp = {k: np.asarray(v) for k, v in setup_inputs().items()}
    ref = np.asarray(reference(**inp))
    got = kernel(**inp)
    err = np.abs(got - ref)
    print("absmax err:", err.max(), "ref absmax:", np.abs(ref).max())
    print("Relative error:", err.max() / np.abs(ref).max())
